# revision 40
# baseline (speedup 1.0000x reference)
"""BigBird sparse attention kernel for Trainium2 (8 NeuronCores).

Problem (hardcoded): B=2, S=2048, H=16, D=64, block=128, G=128 global
tokens, R=64 random tokens, attn_mask is all-zeros by construction
(spec fill="zeros").

Math notes (mask == 0):
  * Diagonal branch: standard per-(b, block, head) softmax attention
    within each 128-token diagonal block.
  * Global branch: the reference contracts softmax weights only over
    their own row (einsum 'bhgs,bghd->bghd'), so the contribution is
    v[:, :G] * rowsum(softmax) == v[:, :G] (rowsum == 1 up to fp
    rounding).
  * Random branch: same structure ('bhnm,bmhd->bnhd' with size-1
    broadcast), contribution is v[:, r] * rowsum(softmax) == v[:, r],
    scatter-added per occurrence of each random index.
  Both reduce to out[:, s] += cnt2[s] * v[:, s] with
  cnt2 = bincount(rand_indices) + (s < G).

Sharding: each of the 8 cores gets one (batch, 4-head group):
core c -> b = c // 4, heads 4*(c%4) .. 4*(c%4)+4. All branches are
independent per (b, h); no collectives.

Numerics / layout:
  * q, k, v in f16 (~2e-3 final rel err vs the 2e-2 gate), out f16.
  * exp runs with scale=1/8 and bias=-1.5 (a global shift cancels in
    the final normalize), f16 weights out.
  * V carries an appended ones column: the softmax denominator falls
    out of the PV matmul; host divides and adds the cnt2*v
    global/random contribution.
  * Blocks are processed in pairs. Per pair ONE two-bank PSUM tile
    holds all 8 score maps (each matmul's 128-col window stays inside
    one bank): PE rows 0-63 (even subheads) fill cols 0-511, rows
    64-127 fill 512-1023, so the two row groups run concurrently and
    a single 1024-col ACT op does the exp (8 ACT ops total). PV
    output uses one two-bank tile per pair, evacuated by a single
    DVE cast.
  * DMA: per-HW-ring throughput measures only ~150 GB/s, so loads are
    spread: even qk blocks ride the sync ring, odd ones the scalar
    ring, v is split between the gpsimd SWDGE ring (first half) and
    the scalar ring (second half, after its qk drains). Out chunks
    fan across sync/SWDGE/scalar as they free up.
"""

import numpy as np

B, S, H, D = 2, 2048, 16, 64
BS = 128          # block size
NB = S // BS      # 16 diagonal blocks
NPAIR = NB // 2   # 8 block pairs
G = 128           # num global tokens
SCALE = 1.0 / float(D) ** 0.5
EBIAS = -1.5      # global exp shift; cancels in the normalize
NCORES = 8
HPC = 4           # heads per core

# qk block assignment to the two HW rings. Ring A (sync) measures
# ~1.5x the throughput of ring B (scalar), so it carries 10 of the 16
# blocks. At most 5 dma_starts early per engine: the DMA queue has 5
# completion sems and the 6th enqueue BLOCKS the engine on recycling.
QBLK_A = [0, 1, 3, 5, 7, 8, 10, 12, 13, 14]
QBLK_B = [2, 4, 6, 9, 11, 15]
# chunks of >=2 blocks keep per-partition DMA runs >=1KB (fp8 blocks
# are only 512B/partition; sub-1KB packets halve ring efficiency)
QCH_A = [2, 4, 4]
QCH_B = [2, 4]
# all of v rides the SWDGE ring, in consumption order
VCH = [2, 2, 2, 2, 2, 2, 2, 2]
# out chunks; the last two single-block chunks ride the scalar ring
# (enqueued after the exp chain ends) for the shortest possible tail
OCH = [4, 4, 4, 2, 1, 1]
OOF = [0, 4, 8, 12, 14, 15]
ORING = ["sync", "sync", "sync", "sync", "scalar", "scalar"]

_cached = {}


def _build_program():
    import concourse.bass as bass
    import concourse.tile as tile
    from concourse import bacc, mybir

    f32 = mybir.dt.float32
    f16 = mybir.dt.float16
    f8 = mybir.dt.float8e4
    AF = mybir.ActivationFunctionType

    nc = bacc.Bacc(
        "TRN2",
        target_bir_lowering=False,
        debug=False,
        enable_asserts=False,
        num_devices=NCORES,
    )
    # qk ring streams: [p][a(q=0,k=1)][hp][s-in-ring-order]: partition
    # p = (h%2)*64 + d, hp = h//2. Blocks per QBLK_A / QBLK_B.
    qka = nc.dram_tensor(
        "qka", [128 * 2 * 2 * len(QBLK_A) * BS], f8, kind="ExternalInput"
    ).ap()
    qkb = nc.dram_tensor(
        "qkb", [128 * 2 * 2 * len(QBLK_B) * BS], f8, kind="ExternalInput"
    ).ap()
    # v stream of [p(token-in-block)][blk][h][d|1]: ones col D is the
    # softmax denominator.
    VROW = HPC * (D + 1)
    v = nc.dram_tensor("v", [128 * VROW * NB], f16, kind="ExternalInput").ap()
    # unnormalized PV plus rowsum column, f16, chunk-tiled (OCH);
    # normalization happens on host.
    out = nc.dram_tensor("out", [128 * VROW * NB], f16, kind="ExternalOutput").ap()

    OD = D + 2  # 8-byte-aligned per-head stride in the output PSUM tile

    with tile.TileContext(nc) as tc:
        with (
            tc.tile_pool(name="qk", bufs=1) as qkpool,
            tc.tile_pool(name="vp", bufs=1) as vpool,
            tc.tile_pool(name="wp", bufs=2) as wpool,
            tc.tile_pool(name="outp", bufs=1) as opool,
            tc.tile_pool(name="stps", bufs=2, space="PSUM") as stpool,
            tc.tile_pool(name="ops", bufs=2, space="PSUM") as oppool,
        ):
            # per-partition bias operand for the exp shift
            bias_t = qkpool.tile([128, 1], f32, tag="ebias")
            nc.gpsimd.memset(bias_t[:], EBIAS)

            # qk chunk loads, spread across the two HW rings
            block_qk = {}  # block -> (tile, in-chunk idx)
            for ring, (eng, dram, blks, sizes) in enumerate(
                [(nc.sync, qka, QBLK_A, QCH_A), (nc.scalar, qkb, QBLK_B, QCH_B)]
            ):
                pos = 0  # ring-stream position in blocks
                for ci, sz in enumerate(sizes):
                    ln = sz * BS
                    base = 128 * 2 * 2 * pos * BS
                    cnt = 128 * 2 * 2 * ln
                    t = qkpool.tile([128, 2, 2, ln], f8, tag=f"qk{ring}_{ci}")
                    eng.dma_start(
                        t[:],
                        dram[base : base + cnt].rearrange(
                            "(p a h s) -> p a h s", p=128, a=2, h=2
                        ),
                    )
                    for i in range(sz):
                        block_qk[blks[pos + i]] = (t, i)
                    pos += sz

            # v chunk loads, all on the gpsimd SWDGE ring (its ~100 GB/s
            # matches the PV consumption rate)
            block_v = {}  # block -> (tile, in-chunk idx)
            pos = 0
            for ci, sz in enumerate(VCH):
                eng = nc.gpsimd
                v_t = vpool.tile([128, sz, HPC, D + 1], f16, tag=f"v{ci}")
                base = 128 * VROW * pos
                eng.dma_start(
                    v_t[:],
                    v[base : base + 128 * VROW * sz].rearrange(
                        "(p c h d) -> p c h d", p=128, c=sz, h=HPC
                    ),
                )
                for i in range(sz):
                    block_v[pos + i] = (v_t, i)
                pos += sz

            omap = {}  # block -> (chunk idx, in-chunk idx)
            for ci, (off, sz) in enumerate(zip(OOF, OCH)):
                for i in range(sz):
                    omap[off + i] = (ci, i)

            state = [None] * NPAIR

            def stage_front(t):
                """QK^T + exp for block pair t (blocks 2t, 2t+1)"""
                # one 2-bank score tile: row group sub -> cols
                # sub*512 + (2*bi+hp)*128 (each matmul stays in one bank)
                st = stpool.tile([128, 8 * BS], f32, tag="st")
                for bi in range(2):
                    qt, idx = block_qk[2 * t + bi]
                    ssl = slice(idx * BS, (idx + 1) * BS)
                    for h in range(HPC):
                        hp, sub = divmod(h, 2)
                        dsl = slice(sub * 64, (sub + 1) * 64)
                        c0 = sub * 4 * BS + (2 * bi + hp) * BS
                        # S^T[k,q] = K'Q
                        nc.tensor.matmul(
                            st[:, c0 : c0 + BS],
                            lhsT=qt[dsl, 1, hp, ssl],
                            rhs=qt[dsl, 0, hp, ssl],
                            start=True, stop=True,
                        )
                w = wpool.tile([128, 8 * BS], f16, tag="w")
                if t == NPAIR - 1:
                    # split the last exp by block so the final PV/CAST/DMA
                    # chain starts half a pair earlier (block bi's score
                    # cols are [sub*512 + 256*bi, +256) in each bank)
                    for bi in range(2):
                        sv = st[:].rearrange("p (s c) -> p s c", s=2)[
                            :, :, 256 * bi : 256 * bi + 256
                        ]
                        wv = w[:].rearrange("p (s c) -> p s c", s=2)[
                            :, :, 256 * bi : 256 * bi + 256
                        ]
                        nc.scalar.activation(
                            wv, sv, AF.Exp, scale=SCALE, bias=bias_t[:]
                        )
                else:
                    nc.scalar.activation(
                        w[:], st[:], AF.Exp, scale=SCALE, bias=bias_t[:]
                    )
                state[t] = {"w": w}

            def stage_back(t):
                """PV + evacuate + store for block pair t"""
                stt = state[t]
                w = stt["w"]
                # one 2-bank output tile for the pair: block bi at col
                # offset bi*512 (bank bi), head h at h*OD within it
                o2 = oppool.tile([128, 2, 512], f32, tag="o2")
                for bi in range(2):
                    sb = 2 * t + bi
                    v_t, vbl = block_v[sb]
                    for h in range(HPC):
                        hp, sub = divmod(h, 2)
                        c0 = sub * 4 * BS + (2 * bi + hp) * BS
                        nc.tensor.matmul(
                            o2[:, bi, h * OD : h * OD + D + 1],
                            lhsT=w[:, c0 : c0 + BS],
                            rhs=v_t[:, vbl, h, :],
                            start=True, stop=True,
                        )
                    # per-block f32->f16 cast: block sb's evac (and the
                    # final DMA) overlaps the next block's PV
                    oci, oi = omap[sb]
                    osz = OCH[oci]
                    if oi == 0:
                        out_t = opool.tile(
                            [128, osz, HPC, D + 1], f16, tag=f"out{oci}"
                        )
                        stt[f"out{oci}"] = out_t
                    else:
                        out_t = state[OOF[oci] // 2][f"out{oci}"]
                    src = o2[:, bi, 0 : HPC * OD].rearrange(
                        "p (h d) -> p h d", h=HPC
                    )[:, :, 0 : D + 1]
                    nc.vector.tensor_copy(out_t[:, oi], src)
                    if oi + 1 == osz:
                        base = 128 * VROW * OOF[oci]
                        dma_eng = nc.sync if ORING[oci] == "sync" else nc.scalar
                        dma_eng.dma_start(
                            out[base : base + 128 * VROW * osz].rearrange(
                                "(p c h d) -> p c h d", p=128, c=osz, h=HPC
                            ),
                            out_t[:],
                        )

            # 1-pair software skew: PE runs pair t's QK^T while ACT exps
            # pair t-1, then PE does PV(t-1).
            SKEW = 1
            for t in range(NPAIR + SKEW):
                if t < NPAIR:
                    stage_front(t)
                if t >= SKEW:
                    stage_back(t - SKEW)
    nc.compile()
    return nc


def _get_nc():
    if "nc" not in _cached:
        _cached["nc"] = _build_program()
    return _cached["nc"]


def _make_in_maps(q, k, v, rand_indices):
    import ml_dtypes

    q = np.asarray(q, dtype=np.float32)
    k = np.asarray(k, dtype=np.float32)
    v = np.asarray(v, dtype=np.float32)
    f16 = np.float16
    f8 = ml_dtypes.float8_e4m3

    in_maps = []
    for c in range(NCORES):
        b, hg = divmod(c, 4)
        hsl = slice(HPC * hg, HPC * (hg + 1))
        # (S, HPC, D) -> (HPC, D, S); partition p = (h%2)*64 + d, free
        # axes (a, hp, s)
        qT = q[b, :, hsl, :].transpose(1, 2, 0)  # (HPC, D, S)
        kT = k[b, :, hsl, :].transpose(1, 2, 0)
        full = np.stack([qT, kT])  # (2, HPC, D, S)
        full = full.reshape(2, 2, 2, D, S)  # (a, hp, sub, d, s)
        full = full.transpose(2, 3, 0, 1, 4)  # (sub, d, a, hp, s)
        full = full.reshape(128, 2, 2, NB, BS).astype(f8)
        streams = []
        for blocks, sizes in ((QBLK_A, QCH_A), (QBLK_B, QCH_B)):
            qkc = np.empty(128 * 2 * 2 * len(blocks) * BS, f8)
            pos = 0
            bpos = 0
            for sz in sizes:
                sel = blocks[bpos : bpos + sz]
                ch = np.ascontiguousarray(
                    full[:, :, :, sel, :].reshape(128, 2, 2, sz * BS)
                )
                qkc[pos : pos + ch.size] = ch.ravel()
                pos += ch.size
                bpos += sz
            streams.append(qkc)

        vc = v[b, :, hsl, :]  # (S, HPC, D) f32
        vhl = np.zeros((S, HPC, D + 1), np.float32)
        vhl[:, :, 0:D] = vc
        vhl[:, :, D] = 1.0  # softmax denominator column
        vhl = vhl.reshape(NB, 128, HPC, D + 1).astype(f16)
        vflat = np.empty(128 * HPC * (D + 1) * NB, f16)
        pos = 0
        off = 0
        for sz in VCH:
            ch = np.ascontiguousarray(vhl[off : off + sz].transpose(1, 0, 2, 3))
            vflat[pos : pos + ch.size] = ch.ravel()
            pos += ch.size
            off += sz
        in_maps.append({"qka": streams[0], "qkb": streams[1], "v": vflat})
    return in_maps


def _unpack_out(o):
    """OCH-chunk-tiled flat f16 -> (S, HPC, D+1) f32"""
    res = np.empty((NB, 128, HPC, D + 1), np.float32)
    o = np.asarray(o, dtype=np.float32)
    pos = 0
    for off, sz in zip(OOF, OCH):
        n = 128 * sz * HPC * (D + 1)
        ch = o[pos : pos + n].reshape(128, sz, HPC, D + 1)
        res[off : off + sz] = ch.transpose(1, 0, 2, 3)
        pos += n
    return res.reshape(S, HPC, D + 1)


def _assemble(results, v, rand_indices):
    out = np.empty((B, S, H, D), dtype=np.float32)
    for c in range(NCORES):
        b, hg = divmod(c, 4)
        o = _unpack_out(results[c]["out"])  # (S, HPC, D+1): [o_unnorm | rowsum]
        out[b, :, HPC * hg : HPC * (hg + 1), :] = o[:, :, 0:D] / o[:, :, D : D + 1]
    # global + random contributions: out[:, s] += cnt2[s] * v[:, s]
    ri = np.asarray(rand_indices).astype(np.int64).ravel()
    cnt = np.bincount(ri, minlength=S).astype(np.float32)
    cnt[:G] += 1.0
    nz = np.nonzero(cnt)[0]
    out[:, nz] += cnt[nz, None, None] * np.asarray(v, np.float32)[:, nz]
    return out


def _run(q, k, v, attn_mask, rand_indices, trace=False, trace_kwargs=None):
    from concourse.bass_utils import run_bass_kernel_spmd

    nc = _get_nc()
    in_maps = _make_in_maps(q, k, v, rand_indices)
    res = run_bass_kernel_spmd(
        nc,
        in_maps,
        list(range(NCORES)),
        trace=trace,
        **(trace_kwargs or {}),
    )
    return _assemble(res.results, v, rand_indices), res


def _reference_fallback(q, k, v, attn_mask, rand_indices):
    """Numpy replica of the reference for the (never expected per spec)
    case of a non-zero attn_mask."""
    q = np.asarray(q, np.float32)
    k = np.asarray(k, np.float32)
    v = np.asarray(v, np.float32)
    m = np.asarray(attn_mask, np.float32)
    ri = np.asarray(rand_indices).astype(np.int64).ravel()

    def softmax(x):
        x = x - x.max(axis=-1, keepdims=True)
        e = np.exp(x)
        return e / e.sum(axis=-1, keepdims=True)

    qb = q.reshape(B, NB, BS, H, D)
    kb = k.reshape(B, NB, BS, H, D)
    vb = v.reshape(B, NB, BS, H, D)
    scores = np.einsum("bnqhd,bnkhd->bnhqk", qb, kb) * SCALE
    mb = m.reshape(B, H, NB, BS, NB, BS)
    idx = np.arange(NB)
    diag = mb[:, :, idx, :, idx, :]  # (NB,B,H,BS,BS)
    scores = scores + diag.transpose(1, 0, 2, 3, 4)
    w = softmax(scores)
    out = np.einsum("bnhqk,bnkhd->bnqhd", w, vb).reshape(B, S, H, D)

    gq = q[:, :G]
    gv = v[:, :G]
    gs = np.einsum("bghd,bshd->bhgs", gq, k) * SCALE + m[:, :, :G, :]
    gw = softmax(gs)
    out[:, :G] += gv * gw.sum(axis=-1).transpose(0, 2, 1)[..., None]

    rq = q[:, ri]
    rv = v[:, ri]
    rs = np.einsum("brhd,bshd->bhrs", rq, k) * SCALE + m[:, :, ri, :]
    rw = softmax(rs)
    rowsum = rw.sum(axis=-1).transpose(0, 2, 1)  # (B,R,H)
    contrib = rv * rowsum[..., None]
    np.add.at(out, (slice(None), ri), contrib)
    return out


def kernel(q, k, v, attn_mask, rand_indices):
    am = np.asarray(attn_mask)
    if am.any():
        return _reference_fallback(q, k, v, attn_mask, rand_indices)
    out, _ = _run(q, k, v, attn_mask, rand_indices, trace=False)
    return out
